# revision 1
# baseline (speedup 1.0000x reference)
"""PWC-Net correlation (nn_CorrBlock) Trainium2 Bass kernel.

Problem: feat1, feat2 [8, 256, 80, 160] f32 -> leaky_relu(corr, 0.1)
  corr[n, d, h, w] = (1/256) * sum_c feat1[n,c,h,w] * feat2p[n,c,h+dy,w+dx]
  d = 9*dy + dx, (dy, dx) in [0..8]^2, feat2p zero-padded by 4.

Strategy (data-parallel, 1 sample per NeuronCore, 8 cores):
  - Cast-load both feature maps to SBUF as bf16, feat2 into a zero-padded
    [88 x 168] layout, channels on partitions (2 chunks of 128).
  - Row-Gram on TensorE: for each (h, w-block of 80), 4 matmuls
    (2 C-chunks x 2 dy-groups) accumulate T0[80, 5*88], T1[80, 4*88] in
    PSUM: T[w, (dy, u)] = sum_c f1[c,h,w] * f2p[c,h+dy,w0+u].
  - Drain PSUM->SBUF on DVE/ACT with *stride-9 interleave* and 1/256
    scale: V[w, hh, 9u+dy].  Each pixel's 81 taps become the contiguous
    run V[w, hh, 9w + (9dx+dy)] .. +81.
  - Diagonal-AP DMA (partition step = pitch+9) gathers those runs into
    X[w, hh, r] (r = 9dx+dy), leaky-relu on DVE, SWDGE cast-DMA to DRAM
    O[pixel, r] f32.
  - Host reorders O -> [N, 81, H, W] (pure channel permutation + transpose).
"""

import sys

sys.path.insert(0, "/opt/trn_rl_repo")
import numpy as np

N, C, H, W = 8, 256, 80, 160
HP, WP = 88, 168  # padded feat2 dims (+4 each side)
NWB, WB = 2, 80  # w-blocks per row
G, NG = 10, 8  # h-group size, groups
SCALE = 1.0 / C

_cache = {}


def _build(repeat=1):
    import concourse.tile as tile
    from concourse import bacc, mybir
    from concourse.ap import AP

    F32, BF16 = mybir.dt.float32, mybir.dt.bfloat16
    nc = bacc.Bacc("TRN2", target_bir_lowering=False, debug=False)
    f1 = nc.dram_tensor("f1", [C, H * W], F32, kind="ExternalInput")
    f2 = nc.dram_tensor("f2", [C, H * W], F32, kind="ExternalInput")
    O = nc.dram_tensor("O", [H * W, 81], F32, kind="ExternalOutput")

    with tile.TileContext(nc) as tc:
        with (
            tc.tile_pool(name="inp", bufs=1) as inp,
            tc.tile_pool(name="work", bufs=2) as work,
            tc.tile_pool(name="ps", bufs=4, space="PSUM") as ps,
        ):
            f1sb, f2sb = [], []
            for cc in range(2):
                t1 = inp.tile([128, H * W], BF16, tag=f"f1_{cc}")
                f1sb.append(t1)
                t2 = inp.tile([128, HP * WP], BF16, tag=f"f2_{cc}")
                a = t2[:]
                pp = a.ap[0][0]
                # zero pads: top 4 rows, bottom 4 rows, left pad of row 4,
                # then fused right(h)+left(h+1) pads of the 80 data rows
                nc.vector.memset(t2[:, 0 : 4 * WP], 0.0)
                nc.vector.memset(t2[:, 84 * WP : 88 * WP], 0.0)
                nc.vector.memset(t2[:, 4 * WP : 4 * WP + 4], 0.0)
                lr = AP(a.tensor, a.offset + 4 * WP + 164, [[pp, 128], [WP, 80], [1, 8]])
                nc.vector.memset(lr, 0.0)
                f2sb.append(t2)
            # loads emitted interleaved in consumer (h-group) order so the
            # first group's full working set (f1 piece g, f2 pieces k=g,g+1,
            # both C-chunks) arrives before lower-priority pieces
            for g in range(NG + 1):
                for cc in range(2):
                    a = f2sb[cc][:]
                    pp = a.ap[0][0]
                    src2 = f2.ap()[128 * cc : 128 * (cc + 1), :].rearrange(
                        "c (h w) -> c h w", h=H
                    )
                    hp_lo, hp_hi = 10 * g, min(10 * g + 10, HP)
                    d_lo, d_hi = max(hp_lo, 4), min(hp_hi, 84)
                    if d_lo < d_hi:
                        dst = AP(
                            a.tensor,
                            a.offset + d_lo * WP + 4,
                            [[pp, 128], [WP, d_hi - d_lo], [1, W]],
                        )
                        nc.gpsimd.dma_start(dst, src2[:, d_lo - 4 : d_hi - 4, :])
                if g < NG:
                    for cc in range(2):
                        fsrc = f1.ap()[128 * cc : 128 * (cc + 1), :]
                        nc.gpsimd.dma_start(
                            f1sb[cc][:][:, g * G * W : (g + 1) * G * W],
                            fsrc[:, g * G * W : (g + 1) * G * W],
                        )

            for _rep in range(repeat):
              for wb in range(NWB):
                  w0 = wb * WB
                  for grp in range(NG):
                      V = work.tile([WB, G * 792], BF16, tag="V")
                      v = V[:]
                      vp = v.ap[0][0]
                      for hh in range(G):
                          h = grp * G + hh
                          T0 = ps.tile([WB, 440], F32, tag="T0")
                          T1 = ps.tile([WB, 352], F32, tag="T1")
                          for cc in range(2):
                              lhsT = f1sb[cc][:, h * W + w0 : h * W + w0 + WB]
                              a2 = f2sb[cc][:]
                              p2 = a2.ap[0][0]
                              rhs0 = AP(
                                  a2.tensor,
                                  a2.offset + h * WP + w0,
                                  [[p2, 128], [WP, 5], [1, 88]],
                              )
                              rhs1 = AP(
                                  a2.tensor,
                                  a2.offset + (h + 5) * WP + w0,
                                  [[p2, 128], [WP, 4], [1, 88]],
                              )
                              nc.tensor.matmul(
                                  T0[:], lhsT, rhs0, start=(cc == 0), stop=(cc == 1)
                              )
                              nc.tensor.matmul(
                                  T1[:], lhsT, rhs1, start=(cc == 0), stop=(cc == 1)
                              )
                          # interleaving drains: V[w, hh, 9u+dy] = T[w, dy, u]/C
                          t0 = T0[:]
                          t0p = t0.ap[0][0]
                          src0 = AP(t0.tensor, t0.offset, [[t0p, WB], [1, 88], [88, 5]])
                          dst0 = AP(
                              v.tensor, v.offset + hh * 792, [[vp, WB], [9, 88], [1, 5]]
                          )
                          nc.scalar.mul(dst0, src0, SCALE)
                          t1p = T1[:]
                          t1pp = t1p.ap[0][0]
                          src1 = AP(t1p.tensor, t1p.offset, [[t1pp, WB], [1, 88], [88, 4]])
                          dst1 = AP(
                              v.tensor,
                              v.offset + hh * 792 + 5,
                              [[vp, WB], [9, 88], [1, 4]],
                          )
                          nc.vector.tensor_scalar_mul(dst1, src1, SCALE)

                      # diagonal extraction: X[w, hh, r] = V[w, hh, 9w + r]
                      X = work.tile([WB, G * 81], BF16, tag="X")
                      x = X[:]
                      xp = x.ap[0][0]
                      vsrc = AP(v.tensor, v.offset, [[vp + 9, WB], [792, G], [1, 81]])
                      xdst = AP(x.tensor, x.offset, [[xp, WB], [81, G], [1, 81]])
                      nc.sync.dma_start(xdst, vsrc)
                      # leaky relu in place
                      nc.vector.scalar_tensor_tensor(
                          x, x, 0.1, x, op0=mybir.AluOpType.mult, op1=mybir.AluOpType.max
                      )
                      # store: O[(grp*G+hh)*W + w0 + w, r] f32
                      osrc = AP(x.tensor, x.offset, [[xp, WB], [81, G], [1, 81]])
                      odst = AP(
                          O.ap().tensor,
                          (grp * G * W + w0) * 81,
                          [[81, WB], [W * 81, G], [1, 81]],
                      )
                      nc.gpsimd.dma_start(odst, osrc)

    nc.compile()
    return nc


def _get_nc(repeat=1):
    key = ("nc", repeat)
    if key not in _cache:
        _cache[key] = _build(repeat)
    return _cache[key]


# channel permutation: device writes r = 9*dx + dy, output wants d = 9*dy + dx
_d = np.arange(81)
_R_OF_D = (_d % 9) * 9 + _d // 9


def _run(feat1, feat2, trace=False):
    from concourse.bass_utils import run_bass_kernel_spmd

    nc = _get_nc()
    in_maps = [
        {
            "f1": np.ascontiguousarray(feat1[i].reshape(C, H * W), dtype=np.float32),
            "f2": np.ascontiguousarray(feat2[i].reshape(C, H * W), dtype=np.float32),
        }
        for i in range(N)
    ]
    res = run_bass_kernel_spmd(nc, in_maps, core_ids=list(range(N)), trace=trace)
    out = np.stack([res.results[i]["O"] for i in range(N)])  # [N, H*W, 81]
    out = out.reshape(N, H, W, 81)[..., _R_OF_D].transpose(0, 3, 1, 2)
    return np.ascontiguousarray(out, dtype=np.float32), res


def kernel(feat1, feat2):
    out, _ = _run(np.asarray(feat1), np.asarray(feat2))
    return out



# revision 9
# speedup vs baseline: 1.5088x; 1.5088x over previous
"""PWC-Net correlation (nn_CorrBlock) Trainium2 Bass kernel — 2D-tile Gram.

Problem: feat1, feat2 [8, 256, 80, 160] f32 -> leaky_relu(corr, 0.1)
  corr[n, d, h, w] = (1/256) * sum_c feat1[n,c,h,w] * feat2p[n,c,h+dy,w+dx]
  d = 9*dy + dx, (dy, dx) in [0..8]^2, feat2p zero-padded by 4.

Strategy (data-parallel, 1 sample per NeuronCore, 8 cores):
  - Cast-load both feature maps to SBUF as bf16 (feat2 zero-padded to
    [88 x 168]), channels on partitions (2 chunks of 128); feat1 is
    pre-scaled in place by 1/256 (exact exponent shift in bf16).
  - 2D pixel tiles of TH x TW = 8 x 16 = 128 pixels. Per tile, the taps
    any pixel needs form one 16 x 24 = 384-tap patch of feat2p. Two
    matmuls (C chunks) compute the full cross product
    T[pixel, tap] = sum_c f1[c, pixel] * f2p[c, tap] in PSUM [128, 384]:
    6 streamed columns per pixel vs ~20 for a row-Gram.
  - Drain+leaky-relu PSUM->SBUF bf16, split ACT (Prelu activation,
    first SA cols) / DVE (scalar_tensor_tensor mult+max, rest).
  - One batched DMA per tile row stores the 10 drained patches
    [128, 3840] bf16 to DRAM (contiguous 768B+ runs).
  - Host extracts each pixel's 9x9 window from its patch (pure gather)
    and casts to f32.
"""

import sys

sys.path.insert(0, "/opt/trn_rl_repo")
import numpy as np

N, C, H, W = 8, 256, 80, 160
HP, WP = 88, 168  # padded feat2 dims (+4 each side)
TH, TW = 8, 16  # pixel tile
PH, PW = TH + 8, TW + 8  # tap patch per tile
PATCH = PH * PW  # 384
BANK = 512  # PSUM bank stride (f32) for bank-aligned matmul outputs
NTI, NTJ = H // TH, W // TW  # 10 x 10 tiles
SA = 280  # ACT drain columns (Prelu); DVE copy-drains PATCH-SA + relus them
G, NG = 10, 8  # h-group size for load interleaving
SCALE = 1.0 / C

_cache = {}


def _build(repeat=1, use_prelu=True):
    import concourse.tile as tile
    from concourse import bacc, mybir
    from concourse.ap import AP

    F32, BF16 = mybir.dt.float32, mybir.dt.bfloat16
    nc = bacc.Bacc("TRN2", target_bir_lowering=False, debug=False)
    f1 = nc.dram_tensor("f1", [C, H * W], F32, kind="ExternalInput")
    f2 = nc.dram_tensor("f2", [C, H * W], F32, kind="ExternalInput")
    O = nc.dram_tensor("O", [NTI * NTJ * 128, PATCH], BF16, kind="ExternalOutput")

    with tile.TileContext(nc) as tc:
        with (
            tc.tile_pool(name="inp", bufs=1) as inp,
            tc.tile_pool(name="work", bufs=2) as work,
            tc.tile_pool(name="ps", bufs=2, space="PSUM") as ps,
        ):
            f1sb, f2sb = [], []
            for cc in range(2):
                t1 = inp.tile([128, H * W], BF16, tag=f"f1_{cc}")
                f1sb.append(t1)
                t2 = inp.tile([128, HP * WP], BF16, tag=f"f2_{cc}")
                a = t2[:]
                pp = a.ap[0][0]
                # zero pads: top 4 rows, bottom 4 rows, left pad of row 4,
                # then fused right(h)+left(h+1) pads of the 80 data rows
                nc.vector.memset(t2[:, 0 : 4 * WP], 0.0)
                nc.vector.memset(t2[:, 84 * WP : 88 * WP], 0.0)
                nc.vector.memset(t2[:, 4 * WP : 4 * WP + 4], 0.0)
                lr = AP(a.tensor, a.offset + 4 * WP + 164, [[pp, 128], [WP, 80], [1, 8]])
                nc.vector.memset(lr, 0.0)
                f2sb.append(t2)
            # loads emitted interleaved in consumer (h-group) order so the
            # first tile rows' working set arrives before later pieces
            for g in range(NG + 1):
                for cc in range(2):
                    a = f2sb[cc][:]
                    pp = a.ap[0][0]
                    src2 = f2.ap()[128 * cc : 128 * (cc + 1), :].rearrange(
                        "c (h w) -> c h w", h=H
                    )
                    hp_lo, hp_hi = 10 * g, min(10 * g + 10, HP)
                    d_lo, d_hi = max(hp_lo, 4), min(hp_hi, 84)
                    if d_lo < d_hi:
                        dst = AP(
                            a.tensor,
                            a.offset + d_lo * WP + 4,
                            [[pp, 128], [WP, d_hi - d_lo], [1, W]],
                        )
                        nc.gpsimd.dma_start(dst, src2[:, d_lo - 4 : d_hi - 4, :])
                if g < NG:
                    for cc in range(2):
                        fsrc = f1.ap()[128 * cc : 128 * (cc + 1), :]
                        sl = f1sb[cc][:][:, g * G * W : (g + 1) * G * W]
                        nc.gpsimd.dma_start(sl, fsrc[:, g * G * W : (g + 1) * G * W])
                        # pre-scale by 1/256 in place (exact in bf16)
                        nc.vector.tensor_scalar_mul(sl, sl, SCALE)

            for _rep in range(repeat):
                for ti in range(NTI):
                    V = work.tile([128, NTJ * PATCH], BF16, tag="V")
                    v = V[:]
                    vp = v.ap[0][0]
                    h0 = ti * TH
                    # process tj in groups of 4 sharing one 4-bank PSUM tile;
                    # matmul outputs are bank-aligned (512 f32 apart) so one
                    # grouped drain op covers the group with a 3D AP
                    for tj0 in range(0, NTJ, 4):
                        ng = min(4, NTJ - tj0)
                        T4 = ps.tile([128, 4 * BANK], F32, tag="T4")
                        t4 = T4[:]
                        tp = t4.ap[0][0]
                        for k in range(ng):
                            tj = tj0 + k
                            w0 = tj * TW
                            for cc in range(2):
                                # f1 host-tiled: tile's 128 pixels contiguous
                                a1 = f1sb[cc][:]
                                p1 = a1.ap[0][0]
                                lhsT = AP(
                                    a1.tensor,
                                    a1.offset + (ti * NTJ + tj) * 128,
                                    [[p1, 128], [1, 128]],
                                )
                                a2 = f2sb[cc][:]
                                p2 = a2.ap[0][0]
                                rhs = AP(
                                    a2.tensor,
                                    a2.offset + h0 * WP + w0,
                                    [[p2, 128], [WP, PH], [1, PW]],
                                )
                                nc.tensor.matmul(
                                    T4[:, k * BANK : k * BANK + PATCH],
                                    lhsT,
                                    rhs,
                                    start=(cc == 0),
                                    stop=(cc == 1),
                                )
                        # grouped drains: ACT Prelu on cols [0,SA), DVE
                        # copy-drain [SA,PATCH) then 4x-mode bf16 relu
                        asrc = AP(t4.tensor, t4.offset, [[tp, 128], [BANK, ng], [1, SA]])
                        adst = AP(
                            v.tensor,
                            v.offset + tj0 * PATCH,
                            [[vp, 128], [PATCH, ng], [1, SA]],
                        )
                        if use_prelu:
                            nc.scalar.activation(
                                adst,
                                asrc,
                                mybir.ActivationFunctionType.Prelu,
                                bias=0.0,
                                scale=1.0,
                                alpha=0.1,
                            )
                        else:
                            nc.scalar.mul(adst, asrc, 1.0)
                        dsrc = AP(
                            t4.tensor, t4.offset + SA, [[tp, 128], [BANK, ng], [1, PATCH - SA]]
                        )
                        ddst = AP(
                            v.tensor,
                            v.offset + tj0 * PATCH + SA,
                            [[vp, 128], [PATCH, ng], [1, PATCH - SA]],
                        )
                        nc.vector.tensor_copy(ddst, dsrc)
                        nc.vector.scalar_tensor_tensor(
                            ddst,
                            ddst,
                            0.1,
                            ddst,
                            op0=mybir.AluOpType.mult,
                            op1=mybir.AluOpType.max,
                        )
                        if not use_prelu:
                            nc.vector.scalar_tensor_tensor(
                                adst,
                                adst,
                                0.1,
                                adst,
                                op0=mybir.AluOpType.mult,
                                op1=mybir.AluOpType.max,
                            )
                    # batched store: 10 patches -> O[(ti*NTJ+tj)*128 + p, :]
                    osrc = AP(v.tensor, v.offset, [[vp, 128], [PATCH, NTJ], [1, PATCH]])
                    odst = AP(
                        O.ap().tensor,
                        ti * NTJ * 128 * PATCH,
                        [[PATCH, 128], [128 * PATCH, NTJ], [1, PATCH]],
                    )
                    nc.sync.dma_start(odst, osrc)

    nc.compile()
    return nc


def _get_nc(repeat=1, use_prelu=True):
    key = ("nc", repeat, use_prelu)
    if key not in _cache:
        _cache[key] = _build(repeat, use_prelu)
    return _cache[key]


def _prep_f1(f1_sample):
    """[C, H, W] -> tile-major [C, NTI*NTJ*128] so each tile's 128 pixels
    are contiguous (matmul weights need a single free dimension)."""
    t = f1_sample.reshape(C, NTI, TH, NTJ, TW).transpose(0, 1, 3, 2, 4)
    return np.ascontiguousarray(t.reshape(C, H * W), dtype=np.float32)


# host-side window gather indices: value = P[.., di, dj, di+dy, dj+dx]
_DI = np.arange(TH)[:, None, None, None]
_DJ = np.arange(TW)[None, :, None, None]
_DY = np.arange(9)[None, None, :, None]
_DX = np.arange(9)[None, None, None, :]


def _unpack(out_raw):
    """[n, NTI*NTJ*128, PATCH] patches -> [n, 81, H, W] f32."""
    n = out_raw.shape[0]
    P = np.asarray(out_raw, dtype=np.float32).reshape(n, NTI, NTJ, TH, TW, PH, PW)
    Gt = P[:, :, :, _DI, _DJ, _DI + _DY, _DJ + _DX]  # [n, NTI, NTJ, TH, TW, 9, 9]
    out = Gt.transpose(0, 5, 6, 1, 3, 2, 4).reshape(n, 81, H, W)
    return np.ascontiguousarray(out)


def _run(feat1, feat2, trace=False):
    from concourse.bass_utils import run_bass_kernel_spmd

    nc = _get_nc()
    in_maps = [
        {
            "f1": _prep_f1(feat1[i]),
            "f2": np.ascontiguousarray(feat2[i].reshape(C, H * W), dtype=np.float32),
        }
        for i in range(N)
    ]
    res = run_bass_kernel_spmd(nc, in_maps, core_ids=list(range(N)), trace=trace)
    out_raw = np.stack([np.asarray(res.results[i]["O"]) for i in range(N)])
    return _unpack(out_raw), res


def kernel(feat1, feat2):
    out, _ = _run(np.asarray(feat1), np.asarray(feat2))
    return out
